# revision 8
# baseline (speedup 1.0000x reference)
"""Multi-head attention (QKV proj + RoPE + softmax attention + output proj)
for Trainium2, tensor-parallel over heads across 8 NeuronCores.

Shapes (hardcoded): hidden_states [2, 2048, 2048], 16 heads x 128 head_dim.
Each core computes 2 heads end-to-end:
  q/k/v column-sharded projections -> RoPE -> scores^T -> exp (bias -4ln2,
  cancels in normalization; fp16-safe) -> key-axis sum via fp16 DVE adds +
  gpsimd partition reduce -> out^T = v^T @ exp^T -> normalize -> row-sharded
  O-projection partial (fp16). Host sums the 8 partial outputs in f32.

All tensors fp16 (full-rate PE, 2x DVE, half DMA/SBUF of f32; ~1e-3 rel).

Device layouts:
  - X^T [2048 hidden, 4096 tokens] fp16 streamed 256-col blocks.
  - q^T/k^T kept [128 d, tokens] per head (contraction on partitions).
  - v kept token-major [tokens, 256] (keys on partitions for out^T matmul).
  - RoPE via sign-folded permutation matmul: tmp = S @ q, then
    q_rot = q*cos + tmp*sin elementwise on DVE.

Schedule: A(b0) -> B/C(b0) with A(b1) matmuls fed into PE idle slots ->
B/C(b1). C tiles for query-block qb are emitted between the two B units of
qb+1 so the softmax-denominator tail never stalls PE. Out DMAs issue from
the (otherwise idle) Pool queue.
"""

import math

import numpy as np

HIDDEN = 2048
NH = 16
HD = 128
B = 2
S = 2048
T = B * S
NCORES = 8
HPC = NH // NCORES  # heads per core
CW = HPC * HD  # per-core projection width (256)
BASE = 10000.0
TB = 256  # phase-A token block
QB = 512  # phase-B query block
NKT = S // 128  # key tiles per batch (16)
NCH = HIDDEN // 128  # contraction chunks (16)
EXP_BIAS = -4.0 * math.log(2.0)  # exp guard; cancels in normalization

_CACHE = {}
import os as _os
VARIANT = _os.environ.get("KVAR", "")


def _kernel_body(tc, aps, repeat=1, phases="ABC", variant=""):
    import concourse.bass as bass  # noqa: F401
    import concourse.bass_isa as bass_isa
    from concourse import mybir

    nc = tc.nc
    f32 = mybir.dt.float32
    flags = set(variant.split(",")) if variant else set()
    f16 = mybir.dt.bfloat16 if "bf16" in flags else mybir.dt.float16
    Act = mybir.ActivationFunctionType

    xt_r = aps["xt"].rearrange("(c p) t -> p c t", p=128)
    wq_r = aps["wq"].rearrange("(c p) m -> p c m", p=128)
    wk_r = aps["wk"].rearrange("(c p) m -> p c m", p=128)
    wv_r = aps["wv"].rearrange("(c p) m -> p c m", p=128)
    wo_r = aps["wo"].rearrange("(h p) n -> p h n", p=128)
    out_ap = aps["out"]

    with (
        tc.tile_pool(name="consts", bufs=1) as consts,
        tc.tile_pool(name="big", bufs=2) as big,
        tc.tile_pool(name="xt", bufs=2) as xtp,
        tc.tile_pool(name="rope", bufs=6) as rope,
        tc.tile_pool(name="expp", bufs=2) as expp,
        tc.tile_pool(name="small", bufs=3) as small,
        tc.tile_pool(name="stage", bufs=3) as stagep,
        tc.tile_pool(name="ps", bufs=6, space="PSUM") as psp,
        tc.tile_pool(name="psb", bufs=2, space="PSUM") as psb,
    ):
        # ---- constants (critical first: wq on SP, first xt issued in body
        # on ACT; everything else behind them) ----
        wq_sb = consts.tile([128, NCH, CW], f16, tag="wq")
        wk_sb = consts.tile([128, NCH, CW], f16, tag="wk")
        wv_sb = consts.tile([128, NCH, CW], f16, tag="wv")
        wo_sb = consts.tile([128, HPC, HIDDEN], f16, tag="wo")
        cos_sb = consts.tile([128, S], f16, tag="cos")
        sin_sb = consts.tile([128, S], f16, tag="sin")
        st_sb = consts.tile([128, 128], f16, tag="st")
        bqk_sb = consts.tile([128, 5], f32, tag="bqk")
        bvb_sb = consts.tile([128, CW], f16, tag="bvb")
        xt0_sb = consts.tile([128, NCH, TB], f16, tag="xt0")
        # wq + first xt tile race on separate queues so the first matmul
        # starts ~3.5us in; everything else queues behind them.
        nc.sync.dma_start(out=wq_sb, in_=wq_r)
        nc.scalar.dma_start(out=xt0_sb, in_=xt_r[:, :, 0:TB])
        nc.sync.dma_start(out=bqk_sb, in_=aps["bqk"])
        nc.scalar.dma_start(out=st_sb, in_=aps["st"])
        nc.scalar.dma_start(out=cos_sb, in_=aps["cosT"])
        nc.scalar.dma_start(out=sin_sb, in_=aps["sinT"])
        nc.sync.dma_start(out=wk_sb, in_=wk_r)
        nc.sync.dma_start(out=wv_sb, in_=wv_r)
        nc.sync.dma_start(out=bvb_sb, in_=aps["bvb"])
        nc.scalar.dma_start(out=wo_sb, in_=wo_r)

        def body(_=None):
            qTs, kTs, vts, oTs = {}, {}, {}, {}

            def emit_A_tb_gen(b, tbl):
                if tbl == 0:
                    qTs[b] = big.tile([128, HPC, S], f16, tag="qT", name=f"qT{b}")
                    kTs[b] = big.tile([128, HPC, S], f16, tag="kT", name=f"kT{b}")
                    vts[b] = big.tile([128, NKT, CW], f16, tag="vtok", name=f"vt{b}")
                qT, kT, vtok = qTs[b], kTs[b], vts[b]
                g0 = b * S + tbl * TB
                s0 = tbl * TB
                if b == 0 and tbl == 0:
                    xt_t = xt0_sb  # preloaded alongside wq at kernel start
                else:
                    xt_t = xtp.tile([128, NCH, TB], f16, tag="xt")
                    nc.sync.dma_start(out=xt_t, in_=xt_r[:, :, g0 : g0 + TB])
                units = []
                for h in range(HPC):
                    for qk, w_sb, bcol, dstT in (
                        (0, wq_sb, h, qT),
                        (1, wk_sb, 2 + h, kT),
                    ):
                        ps = psp.tile([128, TB], f32, tag="ps")
                        for c in range(NCH):
                            nc.tensor.matmul(
                                ps,
                                lhsT=w_sb[:, c, h * HD : (h + 1) * HD],
                                rhs=xt_t[:, c, :],
                                start=(c == 0),
                                stop=(c == NCH - 1),
                            )
                        strt = rope.tile([128, TB], f16, tag="rt")
                        # ACT computes in*scale + bias; q-scale folded into
                        # the pre-scaled bias column on host side.
                        nc.scalar.activation(
                            strt, ps, Act.Identity,
                            bias=bqk_sb[:, bcol : bcol + 1],
                            scale=(1.0 / math.sqrt(HD) if qk == 0 else 1.0),
                        )
                        units.append((strt, dstT, h))
                        yield
                for strt, dstT, h in units:
                    tps = psp.tile([128, TB], f32, tag="ps")
                    nc.tensor.matmul(tps, lhsT=st_sb, rhs=strt,
                                     start=True, stop=True)
                    t1 = rope.tile([128, TB], f16, tag="rt")
                    nc.vector.tensor_mul(t1, strt, cos_sb[:, s0 : s0 + TB])
                    t2 = rope.tile([128, TB], f16, tag="rt")
                    nc.vector.tensor_mul(t2, tps, sin_sb[:, s0 : s0 + TB])
                    nc.vector.tensor_add(dstT[:, h, s0 : s0 + TB], t1, t2)
                yield
                for sub in range(TB // 128):
                    psv = psp.tile([128, CW], f32, tag="ps")
                    for c in range(NCH):
                        nc.tensor.matmul(
                            psv,
                            lhsT=xt_t[:, c, sub * 128 : (sub + 1) * 128],
                            rhs=wv_sb[:, c, :],
                            start=(c == 0),
                            stop=(c == NCH - 1),
                        )
                    nc.vector.tensor_add(
                        vtok[:, tbl * (TB // 128) + sub, :], psv, bvb_sb
                    )
                    yield

            def emit_A_tb(b, tbl):
                for _ in emit_A_tb_gen(b, tbl):
                    pass

            def emit_B_unit(b, h, qb, feeder=None):
                if h == 0 and qb == 0:
                    oTs[b] = big.tile([128, HPC, S], f16, tag="outT", name=f"oT{b}")
                qT, kT, vtok, outT = qTs[b], kTs[b], vts[b], oTs[b]
                q0 = qb * QB
                expT = expp.tile([128, NKT, QB], f16, tag="expT")
                pso = psb.tile([128, QB], f32, tag="pso")
                acc = small.tile([128, QB], f16, tag="acc",
                                 name=f"acc{b}_{h}_{qb}")

                def consume(kt):
                    nc.tensor.matmul(
                        pso,
                        lhsT=vtok[:, kt, h * HD : (h + 1) * HD],
                        rhs=expT[:, kt, :],
                        start=(kt == 0),
                        stop=(kt == NKT - 1),
                    )
                    if kt == 1:
                        nc.vector.tensor_add(acc, expT[:, 0, :], expT[:, 1, :])
                    elif kt > 1:
                        nc.vector.tensor_add(acc, acc, expT[:, kt, :])

                for kt in range(NKT):
                    ps = psp.tile([128, QB], f32, tag="ps")
                    nc.tensor.matmul(
                        ps,
                        lhsT=kT[:, h, kt * 128 : (kt + 1) * 128],
                        rhs=qT[:, h, q0 : q0 + QB],
                        start=True,
                        stop=True,
                    )
                    nc.scalar.activation(expT[:, kt, :], ps, Act.Exp,
                                         bias=bqk_sb[:, 4:5])
                    if kt >= 1:
                        consume(kt - 1)
                    if feeder is not None and kt % 2 == 1:
                        next(feeder, None)
                consume(NKT - 1)
                rbc = small.tile([128, QB], f32, tag="rbc",
                                 name=f"rb{b}_{h}_{qb}")
                nc.gpsimd.partition_all_reduce(
                    rbc, acc, channels=128, reduce_op=bass_isa.ReduceOp.add
                )
                nc.vector.reciprocal(rbc, rbc)
                nc.vector.tensor_mul(outT[:, h, q0 : q0 + QB], pso, rbc)

            def emit_C_tt(b, tt):
                outT = oTs[b]
                r0 = b * S + tt * 128
                for half in range(2):
                    stage = stagep.tile([128, 2, QB], f16, tag="stage")
                    for sub in range(2):
                        nb = half * 2 + sub
                        psn = psp.tile([128, QB], f32, tag="ps")
                        for h in range(HPC):
                            nc.tensor.matmul(
                                psn,
                                lhsT=outT[:, h, tt * 128 : (tt + 1) * 128],
                                rhs=wo_sb[:, h, nb * QB : (nb + 1) * QB],
                                start=(h == 0),
                                stop=(h == HPC - 1),
                            )
                        if sub == 0:
                            nc.vector.tensor_copy(stage[:, sub, :], psn)
                        else:
                            nc.scalar.activation(stage[:, sub, :], psn,
                                                 Act.Copy)
                    oeng = (nc.sync if half == 0 else nc.scalar) \
                        if "edma" in flags else nc.gpsimd
                    oeng.dma_start(
                        out=out_ap[r0 : r0 + 128, half * 1024 : (half + 1) * 1024],
                        in_=stage.rearrange("p n q -> p (n q)"),
                    )

            def emit_C_qb(b, qb):
                for i in range(QB // 128):
                    emit_C_tt(b, qb * (QB // 128) + i)

            NTBB = S // TB  # A blocks per batch (8)
            NQB = S // QB  # B query blocks per batch (4)
            if "B" not in phases:
                for b in range(B):
                    for tbl in range(NTBB):
                        emit_A_tb(b, tbl)
                    st_ = stagep.tile([128, 2, QB], f16, tag="stage")
                    nc.vector.tensor_copy(st_[:, 0, :], qTs[b][:, 0, :QB])
                    nc.gpsimd.dma_start(
                        out=out_ap[b * S : b * S + 128, :QB], in_=st_[:, 0, :]
                    )
                return

            for tbl in range(NTBB):
                emit_A_tb(0, tbl)

            def a_feed(b):
                for tbl in range(NTBB):
                    yield from emit_A_tb_gen(b, tbl)

            if "nofeed" in flags:
                for tbl in range(NTBB):
                    emit_A_tb(1, tbl)
                fd = iter(())
            else:
                fd = a_feed(1)
            for b in range(B):
                feeder = fd if b == 0 else None
                if "C" in phases:
                    # C(qb-1) between the two B units of qb: PE stays busy
                    # through the qb-1 softmax-denominator tail.
                    for qb in range(NQB):
                        emit_B_unit(b, h=0, qb=qb, feeder=feeder)
                        if qb > 0:
                            emit_C_qb(b, qb - 1)
                        emit_B_unit(b, h=1, qb=qb, feeder=feeder)
                    emit_C_qb(b, NQB - 1)
                else:
                    for qb in range(NQB):
                        for h in range(HPC):
                            emit_B_unit(b, h, qb, feeder)
                if b == 0:
                    for _ in fd:
                        pass
            if "C" not in phases:
                for b in range(B):
                    st_ = stagep.tile([128, 2, QB], f16, tag="stage")
                    nc.vector.tensor_copy(st_[:, 0, :], oTs[b][:, 0, :QB])
                    nc.gpsimd.dma_start(
                        out=out_ap[b * S : b * S + 128, :QB], in_=st_[:, 0, :]
                    )

        if repeat == 1:
            body()
        else:
            from concourse import mybir as _mb

            eng_hints = (
                _mb.EngineType.PE, _mb.EngineType.Activation,
                _mb.EngineType.DVE, _mb.EngineType.SP,
                _mb.EngineType.Pool,
            )

            def unrollable_body(iv0, unroll):
                for i in range(unroll):
                    body(iv0 + i)

            tc.For_i_unrolled_general(
                0, repeat, 1, unrollable_body, max_unroll=1,
                hint_engines=eng_hints,
            )


def _build(repeat=1, phases="ABC", variant=None):
    if variant is None:
        variant = VARIANT
    key = ("nc", repeat, phases, variant)
    if key in _CACHE:
        return _CACHE[key]
    import concourse.bacc as bacc
    import concourse.tile as tile
    from concourse import mybir

    f32 = mybir.dt.float32
    f16 = mybir.dt.bfloat16 if "bf16" in (variant.split(",") if variant else []) else mybir.dt.float16

    nc = bacc.Bacc("TRN2", target_bir_lowering=False, debug=False)
    specs = [
        ("xt", [HIDDEN, T], f16, "ExternalInput"),
        ("wq", [HIDDEN, CW], f16, "ExternalInput"),
        ("wk", [HIDDEN, CW], f16, "ExternalInput"),
        ("wv", [HIDDEN, CW], f16, "ExternalInput"),
        ("wo", [CW, HIDDEN], f16, "ExternalInput"),
        ("bqk", [128, 5], f32, "ExternalInput"),
        ("bvb", [128, CW], f16, "ExternalInput"),
        ("cosT", [128, S], f16, "ExternalInput"),
        ("sinT", [128, S], f16, "ExternalInput"),
        ("st", [128, 128], f16, "ExternalInput"),
        ("out", [T, HIDDEN], f16, "ExternalOutput"),
    ]
    aps = {}
    for name, shape, dt_, kind in specs:
        aps[name] = nc.dram_tensor(name, shape, dt_, kind=kind).ap()
    with tile.TileContext(nc) as tc:
        _kernel_body(tc, aps, repeat=repeat, phases=phases, variant=variant)
    nc.compile()
    _CACHE[key] = nc
    return nc


def _host_inputs(hidden_states, Wq, bq, Wk, bk, Wv, bv, Wo):
    if "bf16" in VARIANT.split(","):
        import ml_dtypes
        f16 = ml_dtypes.bfloat16
    else:
        f16 = np.float16

    X = np.asarray(hidden_states, dtype=np.float32).reshape(T, HIDDEN)
    XT = np.ascontiguousarray(X.T.astype(f16))

    inv = 1.0 / (BASE ** (np.arange(0, HD, 2, dtype=np.float32) / HD))
    t = np.arange(S, dtype=np.float32)
    freqs = np.outer(t, inv)  # [S, 64]
    emb = np.concatenate([freqs, freqs], axis=-1)  # [S, 128]
    cosT = np.ascontiguousarray(np.cos(emb).T.astype(f16))  # [128, S]
    sinT = np.ascontiguousarray(np.sin(emb).T.astype(f16))

    # S matrix: tmp = S_ @ q gives tmp[p] = -q[p+64] (p<64), q[p-64] (p>=64)
    # matmul computes lhsT.T @ rhs, so pass st = S_^T.
    S_ = np.zeros((128, 128), dtype=np.float32)
    for p in range(64):
        S_[p, p + 64] = -1.0
        S_[p + 64, p] = 1.0
    st = np.ascontiguousarray(S_.T.astype(f16))

    in_maps = []
    for c in range(NCORES):
        j0 = c * CW
        bq_c = np.asarray(bq[j0 : j0 + CW], dtype=np.float32)
        bk_c = np.asarray(bk[j0 : j0 + CW], dtype=np.float32)
        bv_c = np.asarray(bv[j0 : j0 + CW], dtype=np.float32)
        # ACT computes in*scale + bias, so pre-scale the q bias columns
        qs = 1.0 / math.sqrt(HD)
        eb = np.full(HD, EXP_BIAS, dtype=np.float32)
        bqk = np.stack(
            [bq_c[:HD] * qs, bq_c[HD:] * qs, bk_c[:HD], bk_c[HD:], eb], axis=1
        ).astype(np.float32)  # [128, 5]
        in_maps.append(
            {
                "xt": XT,
                "wq": np.ascontiguousarray(
                    np.asarray(Wq[:, j0 : j0 + CW], dtype=np.float32).astype(f16)
                ),
                "wk": np.ascontiguousarray(
                    np.asarray(Wk[:, j0 : j0 + CW], dtype=np.float32).astype(f16)
                ),
                "wv": np.ascontiguousarray(
                    np.asarray(Wv[:, j0 : j0 + CW], dtype=np.float32).astype(f16)
                ),
                "wo": np.ascontiguousarray(
                    np.asarray(Wo[j0 : j0 + CW, :], dtype=np.float32).astype(f16)
                ),
                "bqk": np.ascontiguousarray(bqk),
                "bvb": np.ascontiguousarray(
                    np.tile(bv_c[None, :], (128, 1)).astype(f16)
                ),
                "cosT": cosT,
                "sinT": sinT,
                "st": st,
            }
        )
    return in_maps


def kernel(hidden_states, Wq, bq, Wk, bk, Wv, bv, Wo):
    from concourse import bass_utils

    nc = _build(repeat=1)
    in_maps = _host_inputs(hidden_states, Wq, bq, Wk, bk, Wv, bv, Wo)
    res = bass_utils.run_bass_kernel_spmd(nc, in_maps, core_ids=list(range(NCORES)))
    acc = res.results[0]["out"].astype(np.float32)
    for c in range(1, NCORES):
        acc = acc + res.results[c]["out"].astype(np.float32)
    return acc.reshape(B, S, HIDDEN)
